# revision 35
# baseline (speedup 1.0000x reference)
"""Trainium2 Bass kernel for nn_Decoder_recon (4-layer weight-shared transformer
decoder with agent-aware dual attention). Data-parallel: 8 samples -> 8 cores.

Self-contained: hardcodes all shapes; only external dep is the Bass toolchain
at /opt/trn_rl_repo.
"""

import sys

sys.path.insert(0, "/opt/trn_rl_repo")

import numpy as np
import ml_dtypes

import concourse.bass as bass
import concourse.tile as tile
from concourse import mybir
from concourse.masks import make_identity

F32 = mybir.dt.float32
BF16 = mybir.dt.bfloat16
F8 = mybir.dt.float8e4
NPBF16 = ml_dtypes.bfloat16
NPF8 = ml_dtypes.float8_e4m3
AF = mybir.ActivationFunctionType
ALU = mybir.AluOpType
DR = mybir.MatmulPerfMode.DoubleRow
WS = 32.0  # fp8 weight pre-scale (power of 2; LN or a 1/WS copy-scale undoes it)

E, H, HD, DFF = 512, 8, 64, 2048
L, LK, S, NA, LF = 384, 256, 8, 32, 12
NL = 4
P = 128
NQ, NKV_SA, NKV_CA, NF, NFF = 3, 3, 2, 4, 16
EPS = 1e-5

# ---------------------------------------------------------------------------
# host-side prep (all SBUF-destined arrays are partition-first: [128, n, w])
# ---------------------------------------------------------------------------


def _pe_table(d_model=E, max_len=200):
    pos = np.arange(max_len, dtype=np.float32)[:, None]
    div = np.exp(
        np.arange(0, d_model, 2, dtype=np.float32) * (-np.log(10000.0) / d_model)
    )
    pe = np.zeros((max_len, d_model), dtype=np.float32)
    pe[:, 0::2] = np.sin(pos * div)
    pe[:, 1::2] = np.cos(pos * div)
    return pe


def _pfirst(a, n, w):
    """[n*128, w] -> [128, n, w] partition-first."""
    return np.ascontiguousarray(
        np.asarray(a, np.float32).reshape(n, P, w).transpose(1, 0, 2)
    )


def _wt_fm(w):
    """[out, in] weight -> lhsT layout [128, in/128, out], bf16."""
    wt = np.ascontiguousarray(np.asarray(w).T)
    n_in = wt.shape[0]
    assert n_in % P == 0, n_in
    return _pfirst(wt, n_in // P, wt.shape[1]).astype(NPBF16)


def _wt_fm8(w, ws=WS):
    """[out, in] weight -> lhsT layout [128, in/128, out], fp8e4 scaled by ws."""
    wt = np.ascontiguousarray(np.asarray(w, np.float32).T) * ws
    n_in = wt.shape[0]
    assert n_in % P == 0, n_in
    return np.clip(_pfirst(wt, n_in // P, wt.shape[1]), -240, 240).astype(NPF8)


def _bias_fm(b):
    b = np.asarray(b, np.float32)
    return _pfirst(b.reshape(-1, 1), b.size // P, 1).astype(np.float32)


def prep(inp):
    """Returns (shared dict name->array, per_core list of dicts)."""
    f32 = lambda x: np.asarray(x, np.float32)
    scale = 1.0 / np.sqrt(HD)
    v = f32(inp["v"])
    z = f32(inp["z"])
    v_enc = f32(inp["v_enc"])

    g = {}
    # folded input embedding: tgt0 = X0 @ wcomb.T + c0
    W1 = f32(inp["pos_fc_w"])[:, :E]
    W2 = f32(inp["pos_fc_w"])[:, E:]
    wcomb = W1 @ f32(inp["input_fc_w"])  # [512, 34]
    pos = np.repeat(_pe_table()[:LF], NA, axis=0)
    c0 = f32(inp["input_fc_b"]) @ W1.T + pos @ W2.T + f32(inp["pos_fc_b"])
    g["c0"] = _pfirst(c0, NQ, E).astype(NPBF16)  # [128, 3, 512] bf16
    wct = np.zeros((P, E), np.float32)
    wct[:34] = wcomb.T
    g["wcombt"] = wct.astype(NPBF16)

    for pfx in ("sa", "ca"):
        ipw, ipb = f32(inp[f"{pfx}_ipw"]), f32(inp[f"{pfx}_ipb"])
        ipw_s, ipb_s = f32(inp[f"{pfx}_ipw_s"]), f32(inp[f"{pfx}_ipb_s"])
        opw, opb = f32(inp[f"{pfx}_opw"]), f32(inp[f"{pfx}_opb"])
        g[f"{pfx}q_wt"] = _wt_fm8(ipw[:E] * scale, 128.0)
        g[f"{pfx}q_b"] = _bias_fm(ipb[:E] * scale)
        g[f"{pfx}k_wt"] = _wt_fm8(ipw[E : 2 * E], 32.0)
        g[f"{pfx}k_b"] = _bias_fm(ipb[E : 2 * E])
        # SA v-projection runs fp8 DoubleRow off the shared x8 tile; CA v
        # stays bf16 (computed once from venct).
        if pfx == "sa":
            g[f"{pfx}v_wt"] = _wt_fm8(ipw[2 * E :])
        else:
            g[f"{pfx}v_wt"] = _wt_fm(ipw[2 * E :])
        g[f"{pfx}qs_wt"] = _wt_fm8(ipw_s[:E] * scale, 128.0)
        g[f"{pfx}qs_b"] = _bias_fm(ipb_s[:E] * scale)
        g[f"{pfx}ks_wt"] = _wt_fm8(ipw_s[E:], 32.0)
        g[f"{pfx}ks_b"] = _bias_fm(ipb_s[E:])
        g[f"{pfx}op_wt"] = _wt_fm8(opw)
        # v-bias folds into output-proj bias (softmax rows sum to 1)
        g[f"{pfx}op_brow"] = (
            (opb + ipb[2 * E :] @ opw.T).reshape(1, E).astype(NPBF16)
        )

    g["lin1_wt"] = _wt_fm8(f32(inp["lin1_w"]))
    g["lin1_b"] = _bias_fm(inp["lin1_b"])
    g["lin2_wt"] = _wt_fm8(f32(inp["lin2_w"]))
    g["lin2_brow"] = f32(inp["lin2_b"]).reshape(1, E).astype(NPBF16)
    g["mlp1_wt"] = _wt_fm8(f32(inp["mlp1_w"]))
    g["mlp1_b"] = _bias_fm(inp["mlp1_b"])
    g["mlp2_wt"] = _wt_fm8(f32(inp["mlp2_w"]))
    g["mlp2_b"] = _bias_fm(inp["mlp2_b"])
    g["outfc_wt"] = _pfirst(f32(inp["out_fc_w"]).T, 2, 2).astype(NPBF16)

    for nm in ("n1", "n2", "n3"):
        gg, bb = f32(inp[f"{nm}_g"]), f32(inp[f"{nm}_b"])
        g[f"{nm}_g"] = np.broadcast_to(gg, (P, E)).astype(np.float32).copy()
        g[f"{nm}_b"] = np.broadcast_to(bb, (P, E)).astype(np.float32).copy()
        g[f"{nm}_trivial"] = bool(np.all(gg == 1.0) and np.all(bb == 0.0))

    venct = np.ascontiguousarray(v_enc[:, 0, :].T)  # [512, 256]
    g["venct"] = _pfirst(venct, NF, LK).astype(NPBF16)

    pp = np.arange(P)[:, None] % NA
    cc = np.arange(L)[None, :] % NA
    # inverted mask: the blend copies INTER scores into the self tile where
    # the pair is NOT same-agent (frees the inter psum tile at blend time)
    g["mself"] = (pp != cc).astype(np.uint8)

    F = (
        f32(inp["out_fc_b"])[None, :]
        + np.tile(v[0, 0], (LF, 1))
        + f32(inp["scene_norm"])[None, :]
    )
    g["fadd"] = _pfirst(F, NQ, 2).astype(np.float32)

    dec_flat = v[0].reshape(L, 2)
    z3 = z.reshape(L, S, -1)
    per_core = []
    for s in range(S):
        x0 = np.concatenate([dec_flat, z3[:, s, :]], axis=-1)  # [384, 34]
        x0t = np.zeros((P, L), np.float32)
        x0t[:34] = x0.T
        per_core.append({"x0t": x0t.astype(NPBF16)})
    return g, per_core


_BIAS_NAMES = ("saq_b", "sak_b", "saqs_b", "saks_b", "caq_b", "cak_b",
               "caqs_b", "caks_b", "lin1_b", "mlp1_b", "mlp2_b")
_ROW_NAMES = ("saop_brow", "caop_brow", "lin2_brow")


def _flags(g):
    bias_nz = tuple((nm, bool(np.any(np.asarray(g[nm]) != 0))) for nm in _BIAS_NAMES)
    row_nz = tuple(
        (nm, bool(np.any(np.asarray(g[nm], np.float32) != 0))) for nm in _ROW_NAMES
    )
    ln_triv = tuple(g[f"{nm}_trivial"] for nm in ("n1", "n2", "n3"))
    return (bias_nz, row_nz, ln_triv)


# ---------------------------------------------------------------------------
# device kernel
# ---------------------------------------------------------------------------

_WEIGHT_SPECS = [
    ("wcombt", (P, E), BF16),
    ("venct", (P, NF, LK), BF16),
    ("mself", (P, L), mybir.dt.uint8),
    ("fadd", (P, NQ, 2), F32),
    ("lin1_wt", (P, NF, DFF), F8),
    ("lin2_wt", (P, NFF, E), F8),
    ("mlp1_wt", (P, NF, E), F8),
    ("mlp2_wt", (P, NF, 256), F8),
    ("outfc_wt", (P, 2, 2), BF16),
] + [
    ("sav_wt", (P, NF, E), F8),
    ("cav_wt", (P, NF, E), BF16),
] + [
    (f"{pfx}{nm}_wt", (P, NF, E), F8)
    for pfx in ("sa", "ca")
    for nm in ("q", "k", "qs", "ks", "op")
]

_BIAS_N = {"lin1_b": NFF, "mlp2_b": 2}
DBG = False



def _split_multi_waits(nc):
    """Walrus codegen allows one sync-wait per instruction; hoist extras onto
    engine-local InstNoOps inserted just before the offending instruction."""
    n_split = 0
    for fn in nc.m.functions:
        for bb in fn.blocks:
            il = bb.instructions
            i = 0
            while i < len(il):
                inst = il[i]
                si = inst.sync_info
                if si is not None and si.on_wait and len(si.on_wait) > 1:
                    waits = list(si.on_wait)
                    for w in waits[:-1]:
                        nop = mybir.InstNoOp(
                            name=nc.get_next_instruction_name(),
                            sync_info=mybir.SyncInfo(on_wait=[w], on_update=[]),
                            engine=inst.engine,
                            bass_nofuse=True,
                        )
                        nc.register_instruction(nop, overwrite=True)
                        il.insert(i, nop)
                        i += 1
                        n_split += 1
                    inst.sync_info = mybir.SyncInfo(
                        on_wait=[waits[-1]], on_update=list(si.on_update)
                    )
                i += 1
    return n_split


def build(flags):
    bias_nz = dict(flags[0])
    row_nz = dict(flags[1])
    ln_triv = flags[2]

    nc = bass.Bass()
    dram = {}
    # DMA issue order follows this declaration order: embed inputs + SA weights
    # first so compute starts while CA/FFN/head weights stream in.
    order = ["x0t_decl", "wcombt", "c0_decl", "mself",
             "sak_wt", "saks_wt", "saq_wt", "saqs_wt", "sav_wt",
             "saop_wt", "venct", "cak_wt", "caks_wt", "caq_wt", "caqs_wt",
             "cav_wt", "caop_wt", "lin1_wt", "lin2_wt", "mlp1_wt", "mlp2_wt",
             "outfc_wt", "fadd"]
    spec_by_name = {nm: (shp, dt) for nm, shp, dt in _WEIGHT_SPECS}
    for nm, shp, dt in _WEIGHT_SPECS:
        dram[nm] = nc.declare_dram_parameter(nm, list(shp), dt, isOutput=False)
    dram["c0"] = nc.declare_dram_parameter("c0", [P, NQ, E], BF16, isOutput=False)
    extra_f32 = []
    for nm, on in bias_nz.items():
        if on:
            extra_f32.append((nm, [P, _BIAS_N.get(nm, NF), 1]))
    for nm, on in row_nz.items():
        if on:
            dram[nm] = nc.declare_dram_parameter(nm, [1, E], BF16, isOutput=False)
    for i, triv in enumerate(ln_triv):
        if not triv:
            extra_f32.append((f"n{i+1}_g", [P, E]))
            extra_f32.append((f"n{i+1}_b", [P, E]))
    for nm, shp in extra_f32:
        dram[nm] = nc.declare_dram_parameter(nm, shp, F32, isOutput=False)
    dram["x0t"] = nc.declare_dram_parameter("x0t", [P, L], BF16, isOutput=False)
    out_dram = nc.declare_dram_parameter("out", [P, NQ, 2], F32, isOutput=True)
    dbg_dram = None
    if DBG:
        dbg_dram = nc.declare_dram_parameter("dbg", [P, 16, NQ, E], F32,
                                             isOutput=True)
    dbg_idx = [0]

    with tile.TileContext(nc) as tc, \
         tc.tile_pool(name="singles", bufs=1) as singles, \
         tc.tile_pool(name="work", bufs=2) as sb, \
         tc.tile_pool(name="expp", bufs=4) as sb3, \
         tc.tile_pool(name="small", bufs=6) as small, \
         tc.tile_pool(name="ps_sc", bufs=2, space="PSUM") as ps_sc, \
         tc.tile_pool(name="ps_mm", bufs=2, space="PSUM") as ps_mm:

        # ---- load inputs (ordered for early compute start)
        W = {}
        x0t = None
        c0_sb = None
        for nm in order:
            if nm == "x0t_decl":
                x0t = singles.tile([P, L], BF16, tag="x0t", name="x0t")
                nc.sync.dma_start(out=x0t, in_=dram["x0t"][:])
            elif nm == "c0_decl":
                c0_sb = singles.tile([P, NQ, E], BF16, tag="c0", name="c0")
                nc.sync.dma_start(out=c0_sb, in_=dram["c0"][:])
            else:
                shp, dt = spec_by_name[nm]
                W[nm] = singles.tile(list(shp), dt, tag=nm, name=nm)
                nc.sync.dma_start(out=W[nm], in_=dram[nm][:])
        for nm, on in row_nz.items():
            if on:
                W[nm] = singles.tile([1, E], BF16, tag=nm, name=nm)
                nc.sync.dma_start(out=W[nm], in_=dram[nm][:])
        for nm, shp in extra_f32:
            W[nm] = singles.tile(shp, F32, tag=nm, name=nm)
            nc.sync.dma_start(out=W[nm], in_=dram[nm][:])

        ident_bf16 = singles.tile([P, P], BF16, tag="idb", name="idb")
        make_identity(nc, ident_bf16)
        # WS-scaled identity: residual matmul partner for fp8 contracts whose
        # psum carries WS*y (LN is scale-invariant, so WS*(x+y) normalizes
        # identically)
        ident_ws = singles.tile([P, P], BF16, tag="idw", name="idw")
        nc.vector.tensor_scalar_mul(out=ident_ws, in0=ident_bf16, scalar1=WS)
        eps_t = singles.tile([P, 1], F32, tag="eps", name="eps")
        nc.vector.memset(eps_t, EPS)
        ones_row = singles.tile([1, P], BF16, tag="ones", name="ones")
        nc.vector.memset(ones_row, 1.0)

        mself = W["mself"]
        # residual stream: three independent bf16 tiles (per token block)
        tgt = [singles.tile([P, E], BF16, tag=f"tgt{i}", name=f"tgt{i}")
               for i in range(NQ)]
        # v_aug buffers (ones column initialized once)
        va_sa = [singles.tile([P, H, 65], BF16, tag=f"va{j}", name=f"va{j}")
                 for j in range(NKV_SA)]
        va_ca = [singles.tile([P, H, 65], BF16, tag=f"vc{j}", name=f"vc{j}")
                 for j in range(NKV_CA)]
        for t in va_sa + va_ca:
            nc.gpsimd.memset(t[:, :, 64:65], 1.0)

        def bias_ap(nm, fo):
            if nm is not None and bias_nz.get(nm, False):
                return W[nm][:, fo, :]
            return 0.0

        def proj_fm(x_fm, wt, n_out, b_nm, tag, relu=False, n_in=NF, width=L,
                    pool=sb, bufs=None, pm_fo0=None):
            """list of n_out bf16 tiles [P, width]: rows of (W @ X.T)."""
            outs = []
            for fo in range(n_out):
                o = pool.tile([P, width], BF16, tag=f"{tag}{fo}",
                              name=f"{tag}{fo}", bufs=bufs)
                if fo == 0 and pm_fo0 is not None:
                    pm = pm_fo0[:, :width]
                else:
                    pm = ps_mm.tile([P, width], F32, tag="mm", name="pm")
                    for ki in range(n_in):
                        nc.tensor.matmul(
                            pm,
                            wt[:, ki, fo * P : (fo + 1) * P],
                            x_fm[ki],
                            start=(ki == 0),
                            stop=(ki == n_in - 1),
                        )
                nc.scalar.activation(
                    out=o, in_=pm, func=AF.Relu if relu else AF.Copy,
                    bias=bias_ap(b_nm, fo),
                )
                outs.append(o)
            return outs

        def transpose_to_fm(first_wt=None, tag="x_fm"):
            """Transpose tgt -> feature-major x_fm tiles. If first_wt is given,
            interleave the transposes with the first projection's fo=0
            accumulation (real matmuls keep the PE HAM warm through the
            transpose burst)."""
            x_fm = []
            pm0 = None
            if first_wt is not None:
                pm0 = ps_mm.tile([P, L], F32, tag="mm", name="pm0")
            for f in range(NF):
                xf = sb.tile([P, L], BF16, tag=f"{tag}{f}", name=f"{tag}{f}")
                pt = ps_mm.tile([P, L], BF16, tag="mm", name="pt")
                for i in range(NQ):
                    nc.tensor.matmul(
                        pt[:, i * P : (i + 1) * P],
                        tgt[i][:, f * P : (f + 1) * P],
                        ident_bf16,
                        is_transpose=True,
                        start=(i == 0),
                        stop=(i == NQ - 1),
                    )
                nc.vector.tensor_copy(out=xf, in_=pt)
                x_fm.append(xf)
                if pm0 is not None:
                    nc.tensor.matmul(
                        pm0, first_wt[:, f, 0:P], xf,
                        start=(f == 0), stop=(f == NF - 1),
                    )
            return x_fm, pm0

        def transpose_to_fm8(tag):
            """Transpose tgt into a single fp8 feature-major tile [P, NF, L]
            (contiguous k-pairs, as DoubleRow lhsT/rhs APs require). The
            psum->sbuf casts alternate DVE/ACT so neither engine serializes
            the burst."""
            xf8 = sb.tile([P, NF, L], F8, tag=tag, name=tag)
            # pts borrow the score-psum banks (free at boundaries)
            _ptags = [("sfA", 2), ("sfB", 2), ("inA", 1), ("inB", 1)]
            pts = [ps_sc.tile([P, L], BF16, tag=_ptags[f][0], name=f"pt{f}",
                              bufs=_ptags[f][1])
                   for f in range(NF)]
            # i-outer: all four f-transposes of token block i unblock the
            # moment LN finishes block i (4 matmuls of PE work per block
            # instead of 2 keeps the PE fed through the boundary)
            for i in range(NQ):
                for f in range(NF):
                    nc.tensor.matmul(
                        pts[f][:, i * P : (i + 1) * P],
                        tgt[i][:, f * P : (f + 1) * P],
                        ident_bf16,
                        is_transpose=True,
                        start=(i == 0),
                        stop=(i == NQ - 1),
                    )
            for f in range(NF):
                if f % 2 == 0:
                    nc.vector.tensor_copy(out=xf8[:, f, :], in_=pts[f])
                else:
                    nc.scalar.copy(out=xf8[:, f, :], in_=pts[f])
            return xf8

        def _psum_store(out_ap, pm, scale, relu, bias, use_dve=False):
            """psum -> sbuf copy with scale (+relu). When there is no bias,
            odd slots go to DVE so ACT and DVE split the burst."""
            if not (isinstance(bias, float) and bias == 0.0):
                nc.scalar.activation(
                    out=out_ap, in_=pm, func=AF.Relu if relu else AF.Copy,
                    bias=bias, scale=scale,
                )
            elif use_dve:
                if relu:
                    nc.vector.tensor_scalar(
                        out=out_ap, in0=pm, scalar1=scale, scalar2=0.0,
                        op0=ALU.mult, op1=ALU.max,
                    )
                elif scale != 1.0:
                    nc.vector.tensor_scalar_mul(out=out_ap, in0=pm, scalar1=scale)
                else:
                    nc.vector.tensor_copy(out=out_ap, in_=pm)
            else:
                nc.scalar.activation(
                    out=out_ap, in_=pm, func=AF.Relu if relu else AF.Copy,
                    bias=0.0, scale=scale,
                )

        def proj_group(x8, specs, width=L):
            """Several fp8 DoubleRow projections off one x8 tile, interleaved
            at fo granularity so downstream consumers of fo=0 slices unblock
            after the first round."""
            outs = [
                sb.tile([P, NF, width], dt, tag=tag, name=tag)
                for (_, _, tag, _, dt, _) in specs
            ]
            for fo in range(NF):
                for si, (wt, b_nm, tag, ws, dt, relu) in enumerate(specs):
                    pm = ps_mm.tile([P, width], F32, tag="mm", name="pm")
                    for kp in range(NF // 2):
                        nc.tensor.matmul(
                            pm,
                            wt[:, 2 * kp : 2 * kp + 2, fo * P : (fo + 1) * P],
                            x8[:, 2 * kp : 2 * kp + 2, :width],
                            start=(kp == 0),
                            stop=(kp == NF // 2 - 1),
                            perf_mode=DR,
                        )
                    _psum_store(
                        outs[si][:, fo, :], pm, 1.0 / ws, relu,
                        bias_ap(b_nm, fo), use_dve=((fo + si) % 2 == 1),
                    )
            return outs

        def proj_dr(x8, wt8, n_out, n_in, b_nm, tag, width=L, out_dt=F8,
                    relu=True, ws=WS):
            """fp8 DoubleRow projection; output a single [P, n_out, width]
            tile. The 1/ws copy-scale undoes the weight pre-scale."""
            dst = sb.tile([P, n_out, width], out_dt, tag=tag, name=tag)
            for fo in range(n_out):
                pm = ps_mm.tile([P, width], F32, tag="mm", name="pm")
                for kp in range(n_in // 2):
                    nc.tensor.matmul(
                        pm,
                        wt8[:, 2 * kp : 2 * kp + 2, fo * P : (fo + 1) * P],
                        x8[:, 2 * kp : 2 * kp + 2, :width],
                        start=(kp == 0),
                        stop=(kp == n_in // 2 - 1),
                        perf_mode=DR,
                    )
                _psum_store(
                    dst[:, fo, :], pm, 1.0 / ws, relu, bias_ap(b_nm, fo),
                    use_dve=(fo % 2 == 1),
                )
            return dst

        def alloc_contract_pms():
            """Contract/LN psum tiles live on the score banks (free by the
            time a contract starts; all score-tile allocations of the
            sublayer precede these, so ring waits resolve at blend time)."""
            _ctags = [("sfA", 2), ("sfB", 2), ("inA", 1)]
            return [
                ps_sc.tile([P, E], F32, tag=_ctags[i][0], name=f"pm{i}",
                           bufs=_ctags[i][1])
                for i in range(NQ)
            ]

        def contract_stage(pms, src8, wt8, kp, first, last):
            """One k-pair round of the token-major fp8 DR contraction; on the
            last round each block's residual matmul immediately follows."""
            for i in range(NQ):
                nc.tensor.matmul(
                    pms[i],
                    src8[:, 2 * kp : 2 * kp + 2, i * P : (i + 1) * P],
                    wt8[:, 2 * kp : 2 * kp + 2, :],
                    start=first,
                    stop=False,
                    perf_mode=DR,
                )
                if last:
                    nc.tensor.matmul(pms[i], ident_ws, tgt[i], start=False,
                                     stop=True)

        def contract_dr(src8, wt8, n_in, brow_nm):
            """fp8 DoubleRow token-major contraction; residual via ident_ws
            keeps the psum uniformly WS-scaled (LN normalizes it away). The
            residual matmul for block i follows block i's last DR matmul so
            pms[i] stops staggered and LN block 0 starts ~1.3us earlier."""
            pms = alloc_contract_pms()
            last = n_in // 2 - 1
            for kp in range(n_in // 2):
                for i in range(NQ):
                    nc.tensor.matmul(
                        pms[i],
                        src8[:, 2 * kp : 2 * kp + 2, i * P : (i + 1) * P],
                        wt8[:, 2 * kp : 2 * kp + 2, :],
                        start=(kp == 0),
                        stop=False,
                        perf_mode=DR,
                    )
                    if kp == last:
                        nc.tensor.matmul(pms[i], ident_ws, tgt[i],
                                         start=False, stop=True)
            return pms

        def ffn(x8):
            """lin1 fo-rounds with lin2 k-pair stages interleaved (lagged 2
            rounds so the h8 stores have landed): one dense PE stream, and
            lin2 finishes ~one round after lin1 instead of fully after."""
            h8 = sb.tile([P, NFF, L], F8, tag="ffh8", name="ffh8")
            pms = None
            for fo in range(NFF):
                pm = ps_mm.tile([P, L], F32, tag="mm", name="pm")
                for kp in range(NF // 2):
                    nc.tensor.matmul(
                        pm,
                        W["lin1_wt"][:, 2 * kp : 2 * kp + 2,
                                     fo * P : (fo + 1) * P],
                        x8[:, 2 * kp : 2 * kp + 2, :],
                        start=(kp == 0),
                        stop=(kp == NF // 2 - 1),
                        perf_mode=DR,
                    )
                _psum_store(
                    h8[:, fo, :], pm, 1.0 / WS, True, bias_ap("lin1_b", fo),
                    use_dve=(fo % 2 == 1),
                )
                if fo >= 3 and fo % 2 == 1 and fo <= 13:
                    kp2 = (fo - 3) // 2
                    if kp2 == 0:
                        pms = alloc_contract_pms()
                    contract_stage(pms, h8, W["lin2_wt"], kp2, kp2 == 0,
                                   False)
                elif fo == 15:
                    contract_stage(pms, h8, W["lin2_wt"], 6, False, False)
            contract_stage(pms, h8, W["lin2_wt"], 7, False, True)
            return pms

        def fill_v_aug(x_fm, wt, va_list):
            for t, va in enumerate(va_list):
                pm = ps_mm.tile([P, E], F32, tag="mm", name=f"vpm{t}")
                for ki in range(NF):
                    nc.tensor.matmul(
                        pm,
                        x_fm[ki][:, t * P : (t + 1) * P],
                        wt[:, ki, :],
                        start=(ki == 0),
                        stop=(ki == NF - 1),
                    )
                nc.scalar.activation(
                    out=va[:, :, 0:64],
                    in_=pm.rearrange("p (h d) -> p h d", d=64),
                    func=AF.Copy,
                )

        def fill_v_aug_dr(x8, wt8, va_list, ws=WS):
            """SA v-projection: fp8 DoubleRow off the shared x8 tile (token
            block t as stationary operand)."""
            for t, va in enumerate(va_list):
                pm = ps_mm.tile([P, E], F32, tag="mm", name=f"vpm{t}")
                for kp in range(NF // 2):
                    nc.tensor.matmul(
                        pm,
                        x8[:, 2 * kp : 2 * kp + 2, t * P : (t + 1) * P],
                        wt8[:, 2 * kp : 2 * kp + 2, :],
                        start=(kp == 0),
                        stop=(kp == NF // 2 - 1),
                        perf_mode=DR,
                    )
                nc.scalar.activation(
                    out=va[:, :, 0:64],
                    in_=pm.rearrange("p (h d) -> p h d", d=64),
                    func=AF.Copy,
                    scale=1.0 / ws,
                )

        def _hs(t, fpair, koff, cols):
            """head-row slice of a list-of-tiles or a single [P,NF,L] tile"""
            if isinstance(t, list):
                return t[fpair][koff : koff + 64, cols]
            return t[koff : koff + 64, fpair, cols]

        def attention(q_t, qs_t, k_fm, ks_fm, v_aug, nkv, causal, tp,
                      op_wt):
            """Returns the contract psum tiles (residual+out-proj applied):
            the out-proj contraction is interleaved into the pair pipeline
            (kp0 once two pairs are out, kp1+residual after the last),
            keeping the PE dense into the LN boundary."""
            # single fp8 feature-major output tile (DoubleRow out-proj lhsT)
            o8 = sb.tile([P, NF, L], F8, tag=f"{tp}o8", name=f"{tp}o8")

            def scores_pair(fp):
                """Both heads of feature pair fp. Score matmuls contract over
                64 partitions only, so the two heads (koff 0 / 64) interleave
                onto alternating PE row-groups and run concurrently. Per-head
                psum tile [P,2,512] f32: self row and inter row land in
                different banks, so the blend never reads and writes the same
                bank."""
                e0 = [sb3.tile([P, L], BF16, tag=f"{tp}exA{j}",
                               name=f"exA{j}") for j in range(nkv)]
                e1 = [sb3.tile([P, L], BF16, tag=f"{tp}exB{j}",
                               name=f"exB{j}") for j in range(nkv)]
                for j in range(nkv):
                    qoff = P * j if causal else 0
                    wdt = L - qoff
                    # single-bank tiles; self tiles are double-buffered and
                    # hold the blend RESULT (inverted mask copies inter into
                    # them), so inter tiles free at blend time and the next
                    # j's matmuls overlap this j's exp
                    sA = ps_sc.tile([P, 512], F32, tag="sfA", name="sA",
                                    bufs=2)
                    sB = ps_sc.tile([P, 512], F32, tag="sfB", name="sB",
                                    bufs=2)
                    iA = ps_sc.tile([P, 512], F32, tag="inA", name="iA",
                                    bufs=1)
                    iB = ps_sc.tile([P, 512], F32, tag="inB", name="iB",
                                    bufs=1)
                    # row-group-alternating issue: h0 self, h1 self,
                    # h0 inter, h1 inter
                    nc.tensor.matmul(
                        sA[:, :wdt],
                        _hs(ks_fm, fp, 0, slice(j * P, (j + 1) * P)),
                        qs_t[0:64, fp, qoff:L],
                        start=True, stop=True,
                    )
                    nc.tensor.matmul(
                        sB[:, :wdt],
                        _hs(ks_fm, fp, 64, slice(j * P, (j + 1) * P)),
                        qs_t[64:128, fp, qoff:L],
                        start=True, stop=True,
                    )
                    nc.tensor.matmul(
                        iA[:, :wdt],
                        _hs(k_fm, fp, 0, slice(j * P, (j + 1) * P)),
                        q_t[0:64, fp, qoff:L],
                        start=True, stop=True,
                    )
                    nc.tensor.matmul(
                        iB[:, :wdt],
                        _hs(k_fm, fp, 64, slice(j * P, (j + 1) * P)),
                        q_t[64:128, fp, qoff:L],
                        start=True, stop=True,
                    )
                    nc.vector.copy_predicated(
                        out=sA[:, :wdt], mask=mself[:, :wdt],
                        data=iA[:, :wdt],
                    )
                    nc.vector.copy_predicated(
                        out=sB[:, :wdt], mask=mself[:, :wdt],
                        data=iB[:, :wdt],
                    )
                    nc.scalar.activation(
                        out=e0[j][:, qoff:L], in_=sA[:, :wdt], func=AF.Exp,
                    )
                    nc.scalar.activation(
                        out=e1[j][:, qoff:L], in_=sB[:, :wdt], func=AF.Exp,
                    )
                    if causal:
                        for ex in (e0[j], e1[j]):
                            for gg in range(1, 4):
                                nc.gpsimd.memset(
                                    ex[32 * gg : 32 * (gg + 1),
                                       qoff : qoff + 32 * gg],
                                    0.0,
                                )
                return (e0, e1)

            def pv_pair(fpair, exps):
                # both heads of the feature pair accumulate into one PSUM
                # bank; single reciprocal + one broadcast multiply writes the
                # normalized token-major pair tile.
                pv = ps_mm.tile([P, NQ, 2, 65], F32, tag="mm", name="pv")
                for i in range(NQ):
                    njs = (i + 1) if causal else nkv
                    for hh in range(2):
                        for j in range(njs):
                            nc.tensor.matmul(
                                pv[:, i, hh, :],
                                exps[hh][j][:, i * P : (i + 1) * P],
                                v_aug[j][:, 2 * fpair + hh, :],
                                start=(j == 0),
                                stop=(j == njs - 1),
                            )
                rec = small.tile([P, NQ, 2, 1], F32, tag="rec", name="rec")
                nc.vector.reciprocal(rec, pv[:, :, :, 64:65])
                otm = sb.tile([P, NQ, P], BF16, tag=f"{tp}otm{fpair}",
                              name=f"otm{fpair}")
                nc.vector.tensor_mul(
                    out=otm.rearrange("p i (h d) -> p i h d", d=64),
                    in0=pv[:, :, :, 0:64],
                    in1=rec[:, :, :, 0:1].broadcast_to([P, NQ, 2, 64]),
                )
                return otm

            def pair_out(fpair, otm):
                ptr = ps_mm.tile([P, L], BF16, tag="mm", name="ptr")
                for i in range(NQ):
                    nc.tensor.matmul(
                        ptr[:, i * P : (i + 1) * P],
                        otm[:, i, :],
                        ident_bf16,
                        is_transpose=True,
                        start=(i == 0),
                        stop=(i == NQ - 1),
                    )
                if fpair % 2 == 0:
                    nc.vector.tensor_copy(out=o8[:, fpair, :], in_=ptr)
                else:
                    nc.scalar.copy(out=o8[:, fpair, :], in_=ptr)

            # software-pipelined at pair granularity, depth 2: pair p's PV
            # trails pair p+2's scores (expp pool bufs=4 keeps 3 pairs live)
            pend = []
            state = {"done": 0, "pms": None}

            def _pop():
                pfp, pexps = pend.pop(0)
                pair_out(pfp, pv_pair(pfp, pexps))
                state["done"] += 1
                if state["done"] == 2:
                    state["pms"] = alloc_contract_pms()
                    contract_stage(state["pms"], o8, op_wt, 0, True, False)
                elif state["done"] == 4:
                    contract_stage(state["pms"], o8, op_wt, 1, False, True)

            for fp in range(4):
                es = scores_pair(fp)
                pend.append((fp, es))
                if len(pend) > 2:
                    _pop()
            while pend:
                _pop()
            return state["pms"]

        def contract_to_tm(src_fm, wt, n_in, brow_nm):
            """Token-major psum tiles; ki-outer so accumulation starts on the
            first available fm tile; residual (tgt) and bias row are folded
            into the same accumulation group on the PE."""
            add_row = row_nz.get(brow_nm, False)
            pms = [ps_mm.tile([P, E], F32, tag="mm", name=f"pm{i}")
                   for i in range(NQ)]
            for ki in range(n_in):
                for i in range(NQ):
                    nc.tensor.matmul(
                        pms[i],
                        src_fm[ki][:, i * P : (i + 1) * P],
                        wt[:, ki, :],
                        start=(ki == 0),
                        stop=False,
                    )
            for i in range(NQ):
                if add_row:
                    nc.tensor.matmul(pms[i], ones_row, W[brow_nm], start=False,
                                     stop=False)
                # residual add on PE: pm += I.T @ tgt
                nc.tensor.matmul(pms[i], ident_bf16, tgt[i], start=False,
                                 stop=True)
            return pms

        def dbg_dump():
            if dbg_dram is not None:
                for i in range(NQ):
                    f32c = small.tile([P, E], F32, tag="dbgc", name="dbgc")
                    nc.vector.tensor_copy(out=f32c, in_=tgt[i])
                    nc.sync.dma_start(out=dbg_dram[:, dbg_idx[0], i, :], in_=f32c)
                dbg_idx[0] += 1

        def residual_ln(pms, ln_idx):
            triv = ln_triv[ln_idx]
            for i in range(NQ):
                stats = small.tile([P, 6], F32, tag="bnst", name="stats")
                nc.vector.bn_stats(stats, pms[i])
                mv = small.tile([P, 2], F32, tag="bnmv", name="mv")
                nc.vector.bn_aggr(mv, stats)
                # rstd = exp(-0.5*ln(var+eps)): ln/exp live in ONE act table
                # set (natural_log_exp_and_others, shared with attention exp)
                # -- a Sqrt here would force a ~2.7us table reload twice per
                # LN boundary.
                lnv = small.tile([P, 1], F32, tag="lnv", name="lnv")
                nc.scalar.activation(out=lnv, in_=mv[:, 1:2], func=AF.Ln,
                                     bias=eps_t)
                rstd = small.tile([P, 1], F32, tag="rstd", name="rstd")
                nc.scalar.activation(out=rstd, in_=lnv, func=AF.Exp,
                                     scale=-0.5)
                # normalize split across ACT (Identity: pm*rstd + (-mu*rstd),
                # same act table as Ln/Exp) and DVE (blocks alternate) so the
                # three per-block chains don't serialize on one engine; each
                # is emitted in halves so the next phase's transposes start
                # on the first half. Shortening this chain keeps the PE
                # boundary stall under the HAM re-throttle window.
                if i == 0:
                    # block 0 is the boundary critical path: DVE direct
                    # (pm-mu)*rstd needs no -mu*rstd precompute
                    for half in range(2):
                        hs = slice(half * (E // 2), (half + 1) * (E // 2))
                        nc.vector.tensor_scalar(
                            out=tgt[i][:, hs], in0=pms[i][:, hs],
                            scalar1=mv[:, 0:1], scalar2=rstd,
                            op0=ALU.subtract, op1=ALU.mult,
                        )
                else:
                    nm = small.tile([P, 1], F32, tag="nmr", name="nm")
                    nc.vector.tensor_scalar(
                        out=nm, in0=mv[:, 0:1], scalar1=rstd, scalar2=-1.0,
                        op0=ALU.mult, op1=ALU.mult,
                    )
                    for half in range(2):
                        hs = slice(half * (E // 2), (half + 1) * (E // 2))
                        nc.scalar.activation(
                            out=tgt[i][:, hs], in_=pms[i][:, hs],
                            func=AF.Identity, bias=nm, scale=rstd,
                        )
                if not triv:
                    nc.vector.tensor_mul(out=tgt[i], in0=tgt[i],
                                         in1=W[f"n{ln_idx+1}_g"])
                    nc.vector.tensor_add(out=tgt[i], in0=tgt[i],
                                         in1=W[f"n{ln_idx+1}_b"])
            dbg_dump()

        # ---- input embedding: tgt = c0 + (X0 @ wcomb.T)
        for i in range(NQ):
            pm = ps_mm.tile([P, E], F32, tag="mm", name="pm")
            nc.tensor.matmul(
                pm, x0t[:, i * P : (i + 1) * P], W["wcombt"], start=True,
                stop=True,
            )
            nc.vector.tensor_add(out=tgt[i], in0=c0_sb[:, i, :], in1=pm)
        dbg_dump()

        # ---- cross-attn K/V/Ks (fixed across layers)
        venct = [W["venct"][:, f, :] for f in range(NF)]
        kc_fm = proj_fm(venct, W["cak_wt"], NF, "cak_b", "kc", width=LK,
                        pool=singles)
        ksc_fm = proj_fm(venct, W["caks_wt"], NF, "caks_b", "ksc", width=LK,
                         pool=singles)
        fill_v_aug(venct, W["cav_wt"], va_ca)

        # ---- decoder layers (shared weights)
        for _layer in range(NL):
            x8 = transpose_to_fm8("sax8")
            k_t, ks_t, q_t, qs_t = proj_group(x8, [
                (W["sak_wt"], "sak_b", "k_t", 32.0, BF16, False),
                (W["saks_wt"], "saks_b", "ks_t", 32.0, BF16, False),
                (W["saq_wt"], "saq_b", "saq_t", 128.0, BF16, False),
                (W["saqs_wt"], "saqs_b", "saqs_t", 128.0, BF16, False),
            ])
            fill_v_aug_dr(x8, W["sav_wt"], va_sa)
            pms = attention(q_t, qs_t, k_t, ks_t, va_sa, NKV_SA, True, "sa",
                            W["saop_wt"])
            residual_ln(pms, 0)

            x8 = transpose_to_fm8("cax8")
            q_t, qs_t = proj_group(x8, [
                (W["caq_wt"], "caq_b", "caq_t", 128.0, BF16, False),
                (W["caqs_wt"], "caqs_b", "caqs_t", 128.0, BF16, False),
            ])
            pms = attention(q_t, qs_t, kc_fm, ksc_fm, va_ca, NKV_CA, False,
                            "ca", W["caop_wt"])
            residual_ln(pms, 1)

            x8 = transpose_to_fm8("ffx8")
            residual_ln(ffn(x8), 2)

        # ---- head MLP
        xm8 = transpose_to_fm8("mlx8")
        h1 = proj_dr(xm8, W["mlp1_wt"], NF, NF, "mlp1_b", "m18")
        h2 = proj_dr(h1, W["mlp2_wt"], 2, NF, "mlp2_b", "m2b", out_dt=BF16)
        for i in range(NQ):
            pm = ps_mm.tile([P, 2], F32, tag="mm", name="pm")
            for ki in range(2):
                nc.tensor.matmul(
                    pm,
                    h2[:, ki, i * P : (i + 1) * P],
                    W["outfc_wt"][:, ki, :],
                    start=(ki == 0),
                    stop=(ki == 1),
                )
            o = small.tile([P, 2], F32, tag="outt", name="o")
            nc.vector.tensor_add(out=o, in0=W["fadd"][:, i, :], in1=pm)
            nc.sync.dma_start(out=out_dram[:, i, :], in_=o)

    _split_multi_waits(nc)
    return nc


# ---------------------------------------------------------------------------
# runner
# ---------------------------------------------------------------------------

_CACHE = {}


def _get_built(flags):
    if flags not in _CACHE:
        _CACHE[flags] = build(flags)
    return _CACHE[flags]


def make_in_maps(g, per_core):
    flags = _flags(g)
    bias_nz, row_nz, ln_triv = dict(flags[0]), dict(flags[1]), flags[2]
    shared = {nm: g[nm] for nm, _, _ in _WEIGHT_SPECS}
    shared["c0"] = g["c0"]
    for nm, on in bias_nz.items():
        if on:
            shared[nm] = g[nm]
    for nm, on in row_nz.items():
        if on:
            shared[nm] = g[nm]
    for i, triv in enumerate(ln_triv):
        if not triv:
            shared[f"n{i+1}_g"] = g[f"n{i+1}_g"]
            shared[f"n{i+1}_b"] = g[f"n{i+1}_b"]
    return flags, [{**shared, **pc} for pc in per_core]


def _postprocess(results):
    outs = []
    for s in range(S):
        o = np.asarray(results[s]["out"], np.float32)  # [128, 3, 2]
        o = o.transpose(1, 0, 2).reshape(L, 2)
        outs.append(o.reshape(LF, NA, 2))
    return np.stack(outs).astype(np.float32)


def run_on_hw(g, per_core, trace=False, **kw):
    from concourse.bass_utils import run_bass_kernel_spmd

    flags, in_maps = make_in_maps(g, per_core)
    nc = _get_built(flags)
    return run_bass_kernel_spmd(nc, in_maps, list(range(S)), trace=trace, **kw)


def kernel(**inputs):
    g, per_core = prep(inputs)
    res = run_on_hw(g, per_core)
    return _postprocess(res.results)

